# revision 34
# baseline (speedup 1.0000x reference)
"""AdditiveAttention (Bahdanau) Trainium2 Bass kernel — trig-expansion v3.

reference:
    Y = tanh(q[:, :, None, :] + k[:, None, :, :])          # [B,Q,K,H]
    scores = einsum("bqkh,h->bqk", Y, w)
    attn = softmax(scores, axis=-1)
    out = einsum("bqk,bkv->bqv", attn, values)             # [B,Q,H]

B=32, Q=256, K=256, H=128.  Data-parallel over batch: 8 cores x 4 batches.

Algorithm: tanh(s) ~= sum_r coef_r sin(om_r s) (weighted LSQ fit over the
input distribution, 5 frequencies {a, 3a, c, 2c, d}, wrms 4.7e-3), so the
score matrix becomes a 10-row-per-h matmul contraction over bf16 trig
feature maps (sin/cos per frequency per side) instead of a 33.5M-element
tanh.  End-to-end rel err ~5.5e-3 incl. bf16 (gate is 2e-2).

Feature construction per side ([128h, 1024pts] tiles):
  - seed a: args fit the HW Sin table (|x|<~pi, verified by probe): direct
    ACT Sin / Sin(+pi/2); 3a via triple angle (sin^3-0.75sin = -sin(3ax)/4).
  - seeds c, d: fp32 magic-number range reduction (+1.5*2^23) to
    f = frac in [-.5,.5]; cos path re-wraps branchlessly via
    fc = (f>0.25)-f and Sin(-2pi fc + pi/2); 2c via double angle.
  - k-side rows carry w_h*coef_r: folded free into spare scalar slots of the
    producing tensor_scalar/scalar_tensor_tensor ops where possible; raw
    ACT sin/cos rows use one tensor_scalar_mul (bf16 4x mode) or a Pool
    broadcast-multiply.

Engine economics (TimelineSim-calibrated): tensor_scalar 327/594ns
(bf16/f32), scalar_tensor_tensor always 1127ns, Pool tensor_tensor 2127ns,
ACT op 1038ns, PE matmul 107ns/row + ldweights.  Squares go to the
otherwise-idle Pool engine; sins/exps/out-scaling to ACT; everything else
DVE.  PSUM: 8 parallel score accumulation groups in 8 distinct banks
(same-bank group interleave is broken on HW — verified); the out-stage
reuses freed banks via the same pool ring.
"""

import os

import numpy as np

B, Q, K, H = 32, 256, 256, 128
NCORES = 8
BPC = B // NCORES
PTS = BPC * Q

# fit X3 {a,3a | c,2c | d} on [-10.9,10.9], weight N(0,sqrt2)+3e-4 floor
SEED_A = 0.26290939635800314
SEED_C = 1.3681225894947517
SEED_D = 1.8829810106831677
COEF = [
    1.2262729945630335,
    0.35021806233529135,
    0.13069888011324796,
    0.030719212594285544,
    0.06172033167075067,
]
MAGIC = float(1.5 * 2**23)

# feature tiles per side: 0 sinA(1) 1 cosA(1) 2 s3(-1/4) 3 c3(1/4)
#   4 sinC(1) 5 cosC(1) 6 s2c(1/2) 7 c2c(1/2) 8 sinD(1) 9 cosD(1)
AMP = [1.0, 1.0, -0.25, 0.25, 1.0, 1.0, 0.5, 0.5, 1.0, 1.0]
ROWS = []
for fi in range(5):
    si, ci = 2 * fi, 2 * fi + 1
    ROWS.append((si, ci, COEF[fi] / (AMP[si] * AMP[ci])))
    ROWS.append((ci, si, COEF[fi] / (AMP[ci] * AMP[si])))
NT = len(ROWS)  # 10

_CACHE: dict = {}


def _build_nc():
    import concourse.bacc as bacc
    import concourse.tile as tile
    from concourse import mybir

    f32 = mybir.dt.float32
    bf16 = mybir.dt.bfloat16
    AF = mybir.ActivationFunctionType
    ALU = mybir.AluOpType
    PI = float(np.pi)

    nc = bacc.Bacc("TRN2", target_bir_lowering=False, debug=False)

    qT_d = nc.dram_tensor("qT", [H, PTS], f32, kind="ExternalInput")
    kT_d = nc.dram_tensor("kT", [H, PTS], f32, kind="ExternalInput")
    vaug_d = nc.dram_tensor("vaug", [128, BPC * 2 * 129], bf16, kind="ExternalInput")
    wa_d = nc.dram_tensor("wa", [128, NT], f32, kind="ExternalInput")
    out_d = nc.dram_tensor("out", [128, BPC * 2 * H], bf16, kind="ExternalOutput")

    with tile.TileContext(nc) as tc:
        with (
            tc.tile_pool(name="const", bufs=1) as cpool,
            tc.tile_pool(name="qf", bufs=1) as qf_pool,
            tc.tile_pool(name="ksw", bufs=1) as ksw_pool,
            tc.tile_pool(name="kraw", bufs=1) as kraw_pool,
            tc.tile_pool(name="tmp", bufs=1) as tmp_pool,
            tc.tile_pool(name="tmpf", bufs=2) as tmpf_pool,
            tc.tile_pool(name="expS", bufs=4) as expS_pool,
            tc.tile_pool(name="osb", bufs=3) as out_pool,
            tc.tile_pool(name="small", bufs=4) as small_pool,
            tc.tile_pool(name="ps", bufs=8, space="PSUM") as ps_pool,
        ):
            kT = cpool.tile([H, PTS], f32, tag="kT")
            nc.sync.dma_start(kT[:], kT_d.ap()[:, :])
            qT = cpool.tile([H, PTS], f32, tag="qT")
            nc.scalar.dma_start(qT[:], qT_d.ap()[:, :])
            wa = cpool.tile([128, NT], f32, tag="wa")
            nc.gpsimd.dma_start(wa[:], wa_d.ap()[:, :])
            vaug = cpool.tile([128, BPC * 2 * 129], bf16, tag="vaug")
            nc.gpsimd.dma_start(vaug[:], vaug_d.ap()[:, :])

            halfpi = cpool.tile([128, 1], f32, tag="halfpi")
            nc.vector.memset(halfpi[:], PI / 2)

            def wcol(t):
                return wa[:, t : t + 1]

            def quad(src, seed, side, u_on_act=False):
                """Range reduction: returns (f, fc) fp32 tiles for one seed."""
                invP = float(seed / (2 * PI))
                u = tmp_pool.tile([H, PTS], f32, name="u")
                if u_on_act:
                    # Copy keeps float-imm bias: u = src*invP + MAGIC on ACT
                    nc.scalar.activation(u[:], src, AF.Copy, bias=MAGIC, scale=invP)
                else:
                    nc.vector.tensor_scalar(u[:], src, invP, MAGIC, ALU.mult, ALU.add)
                n = tmp_pool.tile([H, PTS], f32, name="n")
                nc.vector.tensor_scalar_sub(n[:], u[:], MAGIC)
                f = tmpf_pool.tile([H, PTS], f32, name="f")
                nc.vector.scalar_tensor_tensor(f[:], src, invP, n[:], ALU.mult, ALU.subtract)
                fc = tmpf_pool.tile([H, PTS], f32, name="fc")
                nc.vector.tensor_scalar(
                    fc[:].bitcast(mybir.dt.uint32),
                    f[:].bitcast(mybir.dt.uint32),
                    0x7FFFFFFF,
                    None,
                    ALU.bitwise_and,
                )
                return f, fc

            # ================= K-side range reductions ======================
            kf_c, kfc_c = quad(kT[:], SEED_C, "k", u_on_act=True)
            kf_d, kfc_d = quad(kT[:], SEED_D, "k", u_on_act=True)

            # ================= K side =======================================
            # ACT sins (bf16)
            k_sA = kraw_pool.tile([H, PTS], bf16, name="k_sA")
            nc.scalar.activation(k_sA[:], kT[:], AF.Sin, scale=SEED_A)
            k_cA = kraw_pool.tile([H, PTS], bf16, name="k_cA")
            nc.scalar.activation(k_cA[:], kT[:], AF.Sin, bias=halfpi[:], scale=SEED_A)
            k_sC = kraw_pool.tile([H, PTS], bf16, name="k_sC")
            nc.scalar.activation(k_sC[:], kf_c[:], AF.Sin, scale=2 * PI)
            k_cC = kraw_pool.tile([H, PTS], bf16, name="k_cC")
            nc.scalar.activation(k_cC[:], kfc_c[:], AF.Sin, bias=halfpi[:], scale=-2 * PI)
            k_sD = kraw_pool.tile([H, PTS], bf16, name="k_sD")
            nc.scalar.activation(k_sD[:], kf_d[:], AF.Sin, scale=2 * PI)
            k_cD = kraw_pool.tile([H, PTS], bf16, name="k_cD")
            nc.scalar.activation(k_cD[:], kfc_d[:], AF.Sin, bias=halfpi[:], scale=-2 * PI)

            ksw = [None] * NT

            def ktile(t):
                ksw[t] = ksw_pool.tile([H, PTS], bf16, name=f"ksw{t}")
                return ksw[t]

            # Pool: raw squares only
            k_sA2 = tmp_pool.tile([H, PTS], f32, name="k_sA2")
            nc.gpsimd.tensor_mul(k_sA2[:], k_sA[:], k_sA[:])
            k_cA2 = tmp_pool.tile([H, PTS], f32, name="k_cA2")
            nc.gpsimd.tensor_mul(k_cA2[:], k_cA[:], k_cA[:])
            k_cS2 = tmp_pool.tile([H, PTS], f32, name="k_cS2")
            nc.gpsimd.tensor_mul(k_cS2[:], k_cC[:], k_cC[:])

            qfeat = [None] * NT

            def qtile(i):
                qfeat[i] = qf_pool.tile([H, PTS], bf16, name=f"qf{i}")
                return qfeat[i]

            # DVE: rows t0/t1 as soon as the k seed-a sins land
            nc.vector.tensor_scalar_mul(ktile(0)[:], k_cA[:], wcol(0))
            nc.vector.tensor_scalar_mul(ktile(1)[:], k_sA[:], wcol(1))

            # q-side reductions + sins
            qf_c, qfc_c = quad(qT[:], SEED_C, "q")
            nc.scalar.activation(qtile(0)[:], qT[:], AF.Sin, scale=SEED_A)
            nc.scalar.activation(qtile(1)[:], qT[:], AF.Sin, bias=halfpi[:], scale=SEED_A)
            nc.scalar.activation(qtile(4)[:], qf_c[:], AF.Sin, scale=2 * PI)
            nc.scalar.activation(qtile(5)[:], qfc_c[:], AF.Sin, bias=halfpi[:], scale=-2 * PI)

            # DVE: rows t4/t5
            nc.vector.tensor_scalar_mul(ktile(4)[:], k_cC[:], wcol(4))
            nc.vector.tensor_scalar_mul(ktile(5)[:], k_sC[:], wcol(5))

            # Pool: k s2 product + q squares
            k_s2raw = tmp_pool.tile([H, PTS], bf16, name="k_s2raw")
            nc.gpsimd.tensor_mul(k_s2raw[:], k_sC[:], k_cC[:])
            q_sA2 = tmp_pool.tile([H, PTS], f32, name="q_sA2")
            nc.gpsimd.tensor_mul(q_sA2[:], qfeat[0][:], qfeat[0][:])
            q_cA2 = tmp_pool.tile([H, PTS], f32, name="q_cA2")
            nc.gpsimd.tensor_mul(q_cA2[:], qfeat[1][:], qfeat[1][:])

            # DVE: rows t3/t2 (k triples)
            sAw = tmp_pool.tile([H, PTS], bf16, name="sAw")
            nc.vector.tensor_scalar_mul(sAw[:], k_sA[:], wcol(3))
            cAw = tmp_pool.tile([H, PTS], bf16, name="cAw")
            nc.vector.tensor_scalar_mul(cAw[:], k_cA[:], wcol(2))
            nc.vector.scalar_tensor_tensor(ktile(3)[:], k_sA2[:], 0.75, sAw[:], ALU.subtract, ALU.mult)
            nc.vector.scalar_tensor_tensor(ktile(2)[:], k_cA2[:], 0.75, cAw[:], ALU.subtract, ALU.mult)

            # q-side seed d reduction + sins
            qf_d, qfc_d = quad(qT[:], SEED_D, "q")
            nc.scalar.activation(qtile(8)[:], qf_d[:], AF.Sin, scale=2 * PI)
            nc.scalar.activation(qtile(9)[:], qfc_d[:], AF.Sin, bias=halfpi[:], scale=-2 * PI)

            # Pool: q c-double square, then qf6 product
            q_cS2 = tmp_pool.tile([H, PTS], f32, name="q_cS2")
            nc.gpsimd.tensor_mul(q_cS2[:], qfeat[5][:], qfeat[5][:])
            nc.gpsimd.tensor_mul(qtile(6)[:], qfeat[4][:], qfeat[5][:])

            # DVE: remaining rows
            nc.vector.tensor_scalar_mul(ktile(7)[:], k_s2raw[:], wcol(7))
            nc.vector.tensor_scalar(ktile(6)[:], k_cS2[:], 0.5, wcol(6), ALU.subtract, ALU.mult)
            nc.vector.scalar_tensor_tensor(qtile(2)[:], q_sA2[:], 0.75, qfeat[0][:], ALU.subtract, ALU.mult)
            nc.vector.scalar_tensor_tensor(qtile(3)[:], q_cA2[:], 0.75, qfeat[1][:], ALU.subtract, ALU.mult)
            nc.vector.tensor_scalar_mul(ktile(8)[:], k_cD[:], wcol(8))
            nc.vector.tensor_scalar_mul(ktile(9)[:], k_sD[:], wcol(9))
            nc.vector.tensor_scalar_sub(qtile(7)[:], q_cS2[:], 0.5)

            # ================= scores =======================================
            def psum_bank():
                return ps_pool.tile([128, 2 * Q], f32, name="psb")

            scores_ps = {}
            for b in range(BPC):
                for chunk in range(2):
                    scores_ps[(b, chunk)] = psum_bank()
            MM_ORDER = [0, 1, 4, 5, 2, 3, 6, 8, 9, 7]
            for mi, t in enumerate(MM_ORDER):
                qi, ki, _ = ROWS[t]
                for b in range(BPC):
                    for chunk in range(2):
                        nc.tensor.matmul(
                            scores_ps[(b, chunk)][:, 0:Q],
                            ksw[t][:, b * K + chunk * 128 : b * K + chunk * 128 + 128],
                            qfeat[qi][:, b * Q : (b + 1) * Q],
                            start=(mi == 0),
                            stop=(mi == NT - 1),
                        )

            # ================= softmax + out ================================
            outs = []
            for b in range(BPC):
                eS = {}
                for chunk in range(2):
                    e = expS_pool.tile([128, Q], bf16, name="eS")
                    nc.scalar.activation(e[:], scores_ps[(b, chunk)][:, 0:Q], AF.Exp)
                    eS[chunk] = e
                for qb in range(2):
                    outp = psum_bank()
                    for chunk in range(2):
                        nc.tensor.matmul(
                            outp[:, 0:129],
                            eS[chunk][:, qb * 128 : qb * 128 + 128],
                            vaug[:, (b * 2 + chunk) * 129 : (b * 2 + chunk + 1) * 129],
                            start=(chunk == 0),
                            stop=(chunk == 1),
                        )
                    outs.append((b, qb, outp))
            osb_all = cpool.tile([128, BPC * 2 * H], bf16, tag="osb_all")
            for gi, (b, qb, outp) in enumerate(outs):
                recip = small_pool.tile([128, 1], f32)
                nc.vector.reciprocal(recip[:], outp[:, 128:129])
                g = b * 2 + qb
                dst = osb_all[:, g * H : (g + 1) * H]
                nc.vector.tensor_scalar_mul(dst, outp[:, 0:128], recip[:])
            HALF = BPC * H  # groups 0-3 | 4-7
            nc.sync.dma_start(out_d.ap()[:, 0:HALF], osb_all[:, 0:HALF])
            nc.sync.dma_start(out_d.ap()[:, HALF:], osb_all[:, HALF:])

    nc.compile()
    return nc


def _get_nc():
    if "nc" not in _CACHE:
        _CACHE["nc"] = _build_nc()
    return _CACHE["nc"]


def _prep_core_inputs(queries, keys, values, w, c):
    import ml_dtypes

    bs = slice(c * BPC, (c + 1) * BPC)
    qT = np.ascontiguousarray(
        queries[bs].transpose(2, 0, 1).reshape(H, PTS), dtype=np.float32
    )
    kT = np.ascontiguousarray(
        keys[bs].transpose(2, 0, 1).reshape(H, PTS), dtype=np.float32
    )
    va = np.ones((BPC, 2, 128, 129), dtype=np.float32)
    va[..., :128] = values[bs].reshape(BPC, 2, 128, 128)
    vaug = np.ascontiguousarray(
        va.transpose(2, 0, 1, 3).reshape(128, BPC * 2 * 129)
    ).astype(ml_dtypes.bfloat16)
    wa = np.empty((128, NT), dtype=np.float32)
    for t, (qi, ki, cf) in enumerate(ROWS):
        wa[:, t] = w * np.float32(cf)
    return {"qT": qT, "kT": kT, "vaug": vaug, "wa": wa}


def kernel(queries, keys, values, w):
    from concourse.bass_utils import run_bass_kernel_spmd
    from concourse._compat import axon_active

    if os.environ.get("BASS_TRACE") and axon_active():
        try:
            import antenv.axon_hooks  # noqa: F401
        except ImportError:
            os.environ["BASS_NEVER_TRACE"] = "1"

    queries = np.asarray(queries, dtype=np.float32)
    keys = np.asarray(keys, dtype=np.float32)
    values = np.asarray(values, dtype=np.float32)
    w = np.asarray(w, dtype=np.float32)

    nc = _get_nc()
    in_maps = [_prep_core_inputs(queries, keys, values, w, c) for c in range(NCORES)]
    res = run_bass_kernel_spmd(nc, in_maps, core_ids=list(range(NCORES)))
    _CACHE["last_result"] = res
    outs = []
    for c in range(NCORES):
        o = np.asarray(res.results[c]["out"], dtype=np.float32)  # [128, BPC*2*H]
        o = o.reshape(128, BPC * 2, H).transpose(1, 0, 2).reshape(BPC * Q, H)
        outs.append(o)
    out = np.concatenate(outs, axis=0)
    return out.reshape(B, Q, H)


# revision 35
# speedup vs baseline: 1.0155x; 1.0155x over previous
"""AdditiveAttention (Bahdanau) Trainium2 Bass kernel — trig-expansion v3.

reference:
    Y = tanh(q[:, :, None, :] + k[:, None, :, :])          # [B,Q,K,H]
    scores = einsum("bqkh,h->bqk", Y, w)
    attn = softmax(scores, axis=-1)
    out = einsum("bqk,bkv->bqv", attn, values)             # [B,Q,H]

B=32, Q=256, K=256, H=128.  Data-parallel over batch: 8 cores x 4 batches.

Algorithm: tanh(s) ~= sum_r coef_r sin(om_r s) (weighted LSQ fit over the
input distribution, 5 frequencies {a, 3a, c, 2c, d}, wrms 4.7e-3), so the
score matrix becomes a 10-row-per-h matmul contraction over bf16 trig
feature maps (sin/cos per frequency per side) instead of a 33.5M-element
tanh.  End-to-end rel err ~5.5e-3 incl. bf16 (gate is 2e-2).

Feature construction per side ([128h, 1024pts] tiles):
  - seed a: args fit the HW Sin table (|x|<~pi, verified by probe): direct
    ACT Sin / Sin(+pi/2); 3a via triple angle (sin^3-0.75sin = -sin(3ax)/4).
  - seeds c, d: fp32 magic-number range reduction (+1.5*2^23) to
    f = frac in [-.5,.5]; cos path re-wraps branchlessly via
    fc = (f>0.25)-f and Sin(-2pi fc + pi/2); 2c via double angle.
  - k-side rows carry w_h*coef_r: folded free into spare scalar slots of the
    producing tensor_scalar/scalar_tensor_tensor ops where possible; raw
    ACT sin/cos rows use one tensor_scalar_mul (bf16 4x mode) or a Pool
    broadcast-multiply.

Engine economics (TimelineSim-calibrated): tensor_scalar 327/594ns
(bf16/f32), scalar_tensor_tensor always 1127ns, Pool tensor_tensor 2127ns,
ACT op 1038ns, PE matmul 107ns/row + ldweights.  Squares go to the
otherwise-idle Pool engine; sins/exps/out-scaling to ACT; everything else
DVE.  PSUM: 8 parallel score accumulation groups in 8 distinct banks
(same-bank group interleave is broken on HW — verified); the out-stage
reuses freed banks via the same pool ring.
"""

import os

import numpy as np

B, Q, K, H = 32, 256, 256, 128
NCORES = 8
BPC = B // NCORES
PTS = BPC * Q

# fit X3 {a,3a | c,2c | d} on [-10.9,10.9], weight N(0,sqrt2)+3e-4 floor
SEED_A = 0.26290939635800314
SEED_C = 1.3681225894947517
SEED_D = 1.8829810106831677
COEF = [
    1.2262729945630335,
    0.35021806233529135,
    0.13069888011324796,
    0.030719212594285544,
    0.06172033167075067,
]
MAGIC = float(1.5 * 2**23)

# feature tiles per side: 0 sinA(1) 1 cosA(1) 2 s3(-1/4) 3 c3(1/4)
#   4 sinC(1) 5 cosC(1) 6 s2c(1/2) 7 c2c(1/2) 8 sinD(1) 9 cosD(1)
AMP = [1.0, 1.0, -0.25, 0.25, 1.0, 1.0, 0.5, 0.5, 1.0, 1.0]
ROWS = []
for fi in range(5):
    si, ci = 2 * fi, 2 * fi + 1
    ROWS.append((si, ci, COEF[fi] / (AMP[si] * AMP[ci])))
    ROWS.append((ci, si, COEF[fi] / (AMP[ci] * AMP[si])))
NT = len(ROWS)  # 10

_CACHE: dict = {}


def _build_nc():
    import concourse.bacc as bacc
    import concourse.tile as tile
    from concourse import mybir

    f32 = mybir.dt.float32
    bf16 = mybir.dt.bfloat16
    AF = mybir.ActivationFunctionType
    ALU = mybir.AluOpType
    PI = float(np.pi)

    nc = bacc.Bacc("TRN2", target_bir_lowering=False, debug=False)

    qT_d = nc.dram_tensor("qT", [H, PTS], f32, kind="ExternalInput")
    kT_d = nc.dram_tensor("kT", [H, PTS], f32, kind="ExternalInput")
    vaug_d = nc.dram_tensor("vaug", [128, BPC * 2 * 129], bf16, kind="ExternalInput")
    wa_d = nc.dram_tensor("wa", [128, NT], f32, kind="ExternalInput")
    out_d = nc.dram_tensor("out", [128, BPC * 2 * H], bf16, kind="ExternalOutput")

    with tile.TileContext(nc) as tc:
        with (
            tc.tile_pool(name="const", bufs=1) as cpool,
            tc.tile_pool(name="qf", bufs=1) as qf_pool,
            tc.tile_pool(name="ksw", bufs=1) as ksw_pool,
            tc.tile_pool(name="kraw", bufs=1) as kraw_pool,
            tc.tile_pool(name="tmp", bufs=1) as tmp_pool,
            tc.tile_pool(name="tmpf", bufs=2) as tmpf_pool,
            tc.tile_pool(name="expS", bufs=4) as expS_pool,
            tc.tile_pool(name="osb", bufs=3) as out_pool,
            tc.tile_pool(name="small", bufs=4) as small_pool,
            tc.tile_pool(name="ps", bufs=8, space="PSUM") as ps_pool,
        ):
            kT = cpool.tile([H, PTS], f32, tag="kT")
            nc.sync.dma_start(kT[:], kT_d.ap()[:, :])
            qT = cpool.tile([H, PTS], f32, tag="qT")
            nc.scalar.dma_start(qT[:], qT_d.ap()[:, :])
            wa = cpool.tile([128, NT], f32, tag="wa")
            nc.gpsimd.dma_start(wa[:], wa_d.ap()[:, :])
            vaug = cpool.tile([128, BPC * 2 * 129], bf16, tag="vaug")
            nc.gpsimd.dma_start(vaug[:], vaug_d.ap()[:, :])

            halfpi = cpool.tile([128, 1], f32, tag="halfpi")
            nc.vector.memset(halfpi[:], PI / 2)

            def wcol(t):
                return wa[:, t : t + 1]

            def quad(src, seed, side, u_on_act=False):
                """Range reduction: returns (f, fc) fp32 tiles for one seed."""
                invP = float(seed / (2 * PI))
                u = tmp_pool.tile([H, PTS], f32, name="u")
                if u_on_act:
                    # Copy keeps float-imm bias: u = src*invP + MAGIC on ACT
                    nc.scalar.activation(u[:], src, AF.Copy, bias=MAGIC, scale=invP)
                else:
                    nc.vector.tensor_scalar(u[:], src, invP, MAGIC, ALU.mult, ALU.add)
                n = tmp_pool.tile([H, PTS], f32, name="n")
                nc.vector.tensor_scalar_sub(n[:], u[:], MAGIC)
                f = tmpf_pool.tile([H, PTS], f32, name="f")
                nc.vector.scalar_tensor_tensor(f[:], src, invP, n[:], ALU.mult, ALU.subtract)
                fc = tmpf_pool.tile([H, PTS], f32, name="fc")
                nc.vector.tensor_scalar(
                    fc[:].bitcast(mybir.dt.uint32),
                    f[:].bitcast(mybir.dt.uint32),
                    0x7FFFFFFF,
                    None,
                    ALU.bitwise_and,
                )
                return f, fc

            # ================= K-side range reductions ======================
            kf_c, kfc_c = quad(kT[:], SEED_C, "k", u_on_act=True)
            kf_d, kfc_d = quad(kT[:], SEED_D, "k", u_on_act=True)

            # ================= K side =======================================
            # ACT sins (bf16)
            k_sA = kraw_pool.tile([H, PTS], bf16, name="k_sA")
            nc.scalar.activation(k_sA[:], kT[:], AF.Sin, scale=SEED_A)
            k_cA = kraw_pool.tile([H, PTS], bf16, name="k_cA")
            nc.scalar.activation(k_cA[:], kT[:], AF.Sin, bias=halfpi[:], scale=SEED_A)
            k_sC = kraw_pool.tile([H, PTS], bf16, name="k_sC")
            nc.scalar.activation(k_sC[:], kf_c[:], AF.Sin, scale=2 * PI)
            k_cC = kraw_pool.tile([H, PTS], bf16, name="k_cC")
            nc.scalar.activation(k_cC[:], kfc_c[:], AF.Sin, bias=halfpi[:], scale=-2 * PI)
            k_sD = kraw_pool.tile([H, PTS], bf16, name="k_sD")
            nc.scalar.activation(k_sD[:], kf_d[:], AF.Sin, scale=2 * PI)
            k_cD = kraw_pool.tile([H, PTS], bf16, name="k_cD")
            nc.scalar.activation(k_cD[:], kfc_d[:], AF.Sin, bias=halfpi[:], scale=-2 * PI)

            ksw = [None] * NT

            def ktile(t):
                ksw[t] = ksw_pool.tile([H, PTS], bf16, name=f"ksw{t}")
                return ksw[t]

            # Pool: raw squares only
            k_sA2 = tmp_pool.tile([H, PTS], f32, name="k_sA2")
            nc.gpsimd.tensor_mul(k_sA2[:], k_sA[:], k_sA[:])
            k_cA2 = tmp_pool.tile([H, PTS], f32, name="k_cA2")
            nc.gpsimd.tensor_mul(k_cA2[:], k_cA[:], k_cA[:])
            k_cS2 = tmp_pool.tile([H, PTS], f32, name="k_cS2")
            nc.gpsimd.tensor_mul(k_cS2[:], k_cC[:], k_cC[:])

            qfeat = [None] * NT

            def qtile(i):
                qfeat[i] = qf_pool.tile([H, PTS], bf16, name=f"qf{i}")
                return qfeat[i]

            # DVE: rows t0/t1 as soon as the k seed-a sins land
            nc.vector.tensor_scalar_mul(ktile(0)[:], k_cA[:], wcol(0))
            nc.vector.tensor_scalar_mul(ktile(1)[:], k_sA[:], wcol(1))

            # q-side reductions + sins
            qf_c, qfc_c = quad(qT[:], SEED_C, "q", u_on_act=True)
            nc.scalar.activation(qtile(0)[:], qT[:], AF.Sin, scale=SEED_A)
            nc.scalar.activation(qtile(1)[:], qT[:], AF.Sin, bias=halfpi[:], scale=SEED_A)
            nc.scalar.activation(qtile(4)[:], qf_c[:], AF.Sin, scale=2 * PI)
            nc.scalar.activation(qtile(5)[:], qfc_c[:], AF.Sin, bias=halfpi[:], scale=-2 * PI)

            # DVE: rows t4/t5
            nc.vector.tensor_scalar_mul(ktile(4)[:], k_cC[:], wcol(4))
            nc.vector.tensor_scalar_mul(ktile(5)[:], k_sC[:], wcol(5))

            # Pool: k s2 product + q squares
            k_s2raw = tmp_pool.tile([H, PTS], bf16, name="k_s2raw")
            nc.gpsimd.tensor_mul(k_s2raw[:], k_sC[:], k_cC[:])
            q_sA2 = tmp_pool.tile([H, PTS], f32, name="q_sA2")
            nc.gpsimd.tensor_mul(q_sA2[:], qfeat[0][:], qfeat[0][:])
            q_cA2 = tmp_pool.tile([H, PTS], f32, name="q_cA2")
            nc.gpsimd.tensor_mul(q_cA2[:], qfeat[1][:], qfeat[1][:])

            # DVE: rows t3/t2 (k triples)
            sAw = tmp_pool.tile([H, PTS], bf16, name="sAw")
            nc.vector.tensor_scalar_mul(sAw[:], k_sA[:], wcol(3))
            cAw = tmp_pool.tile([H, PTS], bf16, name="cAw")
            nc.vector.tensor_scalar_mul(cAw[:], k_cA[:], wcol(2))
            nc.vector.scalar_tensor_tensor(ktile(3)[:], k_sA2[:], 0.75, sAw[:], ALU.subtract, ALU.mult)
            nc.vector.scalar_tensor_tensor(ktile(2)[:], k_cA2[:], 0.75, cAw[:], ALU.subtract, ALU.mult)

            # q-side seed d reduction + sins
            qf_d, qfc_d = quad(qT[:], SEED_D, "q", u_on_act=True)
            nc.scalar.activation(qtile(8)[:], qf_d[:], AF.Sin, scale=2 * PI)
            nc.scalar.activation(qtile(9)[:], qfc_d[:], AF.Sin, bias=halfpi[:], scale=-2 * PI)

            # Pool: q c-double square, then qf6 product
            q_cS2 = tmp_pool.tile([H, PTS], f32, name="q_cS2")
            nc.gpsimd.tensor_mul(q_cS2[:], qfeat[5][:], qfeat[5][:])
            nc.gpsimd.tensor_mul(qtile(6)[:], qfeat[4][:], qfeat[5][:])

            # DVE: remaining rows
            nc.vector.tensor_scalar_mul(ktile(7)[:], k_s2raw[:], wcol(7))
            nc.vector.tensor_scalar(ktile(6)[:], k_cS2[:], 0.5, wcol(6), ALU.subtract, ALU.mult)
            nc.vector.scalar_tensor_tensor(qtile(2)[:], q_sA2[:], 0.75, qfeat[0][:], ALU.subtract, ALU.mult)
            nc.vector.scalar_tensor_tensor(qtile(3)[:], q_cA2[:], 0.75, qfeat[1][:], ALU.subtract, ALU.mult)
            nc.vector.tensor_scalar_mul(ktile(8)[:], k_cD[:], wcol(8))
            nc.vector.tensor_scalar_mul(ktile(9)[:], k_sD[:], wcol(9))
            nc.vector.tensor_scalar_sub(qtile(7)[:], q_cS2[:], 0.5)

            # ================= scores =======================================
            def psum_bank():
                return ps_pool.tile([128, 2 * Q], f32, name="psb")

            scores_ps = {}
            for b in range(BPC):
                for chunk in range(2):
                    scores_ps[(b, chunk)] = psum_bank()
            MM_ORDER = [0, 1, 4, 5, 2, 3, 6, 8, 9, 7]
            for mi, t in enumerate(MM_ORDER):
                qi, ki, _ = ROWS[t]
                for b in range(BPC):
                    for chunk in range(2):
                        nc.tensor.matmul(
                            scores_ps[(b, chunk)][:, 0:Q],
                            ksw[t][:, b * K + chunk * 128 : b * K + chunk * 128 + 128],
                            qfeat[qi][:, b * Q : (b + 1) * Q],
                            start=(mi == 0),
                            stop=(mi == NT - 1),
                        )

            # ================= softmax + out ================================
            outs = []
            for b in range(BPC):
                eS = {}
                for chunk in range(2):
                    e = expS_pool.tile([128, Q], bf16, name="eS")
                    nc.scalar.activation(e[:], scores_ps[(b, chunk)][:, 0:Q], AF.Exp)
                    eS[chunk] = e
                for qb in range(2):
                    outp = psum_bank()
                    for chunk in range(2):
                        nc.tensor.matmul(
                            outp[:, 0:129],
                            eS[chunk][:, qb * 128 : qb * 128 + 128],
                            vaug[:, (b * 2 + chunk) * 129 : (b * 2 + chunk + 1) * 129],
                            start=(chunk == 0),
                            stop=(chunk == 1),
                        )
                    outs.append((b, qb, outp))
            osb_all = cpool.tile([128, BPC * 2 * H], bf16, tag="osb_all")
            for gi, (b, qb, outp) in enumerate(outs):
                recip = small_pool.tile([128, 1], f32)
                nc.vector.reciprocal(recip[:], outp[:, 128:129])
                g = b * 2 + qb
                dst = osb_all[:, g * H : (g + 1) * H]
                nc.vector.tensor_scalar_mul(dst, outp[:, 0:128], recip[:])
            HALF = BPC * H  # groups 0-3 | 4-7
            nc.sync.dma_start(out_d.ap()[:, 0:HALF], osb_all[:, 0:HALF])
            nc.sync.dma_start(out_d.ap()[:, HALF:], osb_all[:, HALF:])

    nc.compile()
    return nc


def _get_nc():
    if "nc" not in _CACHE:
        _CACHE["nc"] = _build_nc()
    return _CACHE["nc"]


def _prep_core_inputs(queries, keys, values, w, c):
    import ml_dtypes

    bs = slice(c * BPC, (c + 1) * BPC)
    qT = np.ascontiguousarray(
        queries[bs].transpose(2, 0, 1).reshape(H, PTS), dtype=np.float32
    )
    kT = np.ascontiguousarray(
        keys[bs].transpose(2, 0, 1).reshape(H, PTS), dtype=np.float32
    )
    va = np.ones((BPC, 2, 128, 129), dtype=np.float32)
    va[..., :128] = values[bs].reshape(BPC, 2, 128, 128)
    vaug = np.ascontiguousarray(
        va.transpose(2, 0, 1, 3).reshape(128, BPC * 2 * 129)
    ).astype(ml_dtypes.bfloat16)
    wa = np.empty((128, NT), dtype=np.float32)
    for t, (qi, ki, cf) in enumerate(ROWS):
        wa[:, t] = w * np.float32(cf)
    return {"qT": qT, "kT": kT, "vaug": vaug, "wa": wa}


def kernel(queries, keys, values, w):
    from concourse.bass_utils import run_bass_kernel_spmd
    from concourse._compat import axon_active

    if os.environ.get("BASS_TRACE") and axon_active():
        try:
            import antenv.axon_hooks  # noqa: F401
        except ImportError:
            os.environ["BASS_NEVER_TRACE"] = "1"

    queries = np.asarray(queries, dtype=np.float32)
    keys = np.asarray(keys, dtype=np.float32)
    values = np.asarray(values, dtype=np.float32)
    w = np.asarray(w, dtype=np.float32)

    nc = _get_nc()
    in_maps = [_prep_core_inputs(queries, keys, values, w, c) for c in range(NCORES)]
    res = run_bass_kernel_spmd(nc, in_maps, core_ids=list(range(NCORES)))
    _CACHE["last_result"] = res
    outs = []
    for c in range(NCORES):
        o = np.asarray(res.results[c]["out"], dtype=np.float32)  # [128, BPC*2*H]
        o = o.reshape(128, BPC * 2, H).transpose(1, 0, 2).reshape(BPC * Q, H)
        outs.append(o)
    out = np.concatenate(outs, axis=0)
    return out.reshape(B, Q, H)


# revision 36
# speedup vs baseline: 1.0778x; 1.0614x over previous
"""AdditiveAttention (Bahdanau) Trainium2 Bass kernel — trig-expansion v3.

reference:
    Y = tanh(q[:, :, None, :] + k[:, None, :, :])          # [B,Q,K,H]
    scores = einsum("bqkh,h->bqk", Y, w)
    attn = softmax(scores, axis=-1)
    out = einsum("bqk,bkv->bqv", attn, values)             # [B,Q,H]

B=32, Q=256, K=256, H=128.  Data-parallel over batch: 8 cores x 4 batches.

Algorithm: tanh(s) ~= sum_r coef_r sin(om_r s) (weighted LSQ fit over the
input distribution, 5 frequencies {a, 3a, c, 2c, d}, wrms 4.7e-3), so the
score matrix becomes a 10-row-per-h matmul contraction over bf16 trig
feature maps (sin/cos per frequency per side) instead of a 33.5M-element
tanh.  End-to-end rel err ~5.5e-3 incl. bf16 (gate is 2e-2).

Feature construction per side ([128h, 1024pts] tiles):
  - seed a: args fit the HW Sin table (|x|<~pi, verified by probe): direct
    ACT Sin / Sin(+pi/2); 3a via triple angle (sin^3-0.75sin = -sin(3ax)/4).
  - seeds c, d: fp32 magic-number range reduction (+1.5*2^23) to
    f = frac in [-.5,.5]; cos path re-wraps branchlessly via
    fc = (f>0.25)-f and Sin(-2pi fc + pi/2); 2c via double angle.
  - k-side rows carry w_h*coef_r: folded free into spare scalar slots of the
    producing tensor_scalar/scalar_tensor_tensor ops where possible; raw
    ACT sin/cos rows use one tensor_scalar_mul (bf16 4x mode) or a Pool
    broadcast-multiply.

Engine economics (TimelineSim-calibrated): tensor_scalar 327/594ns
(bf16/f32), scalar_tensor_tensor always 1127ns, Pool tensor_tensor 2127ns,
ACT op 1038ns, PE matmul 107ns/row + ldweights.  Squares go to the
otherwise-idle Pool engine; sins/exps/out-scaling to ACT; everything else
DVE.  PSUM: 8 parallel score accumulation groups in 8 distinct banks
(same-bank group interleave is broken on HW — verified); the out-stage
reuses freed banks via the same pool ring.
"""

import os

import numpy as np

B, Q, K, H = 32, 256, 256, 128
NCORES = 8
BPC = B // NCORES
PTS = BPC * Q

# fit X3 {a,3a | c,2c | d} on [-10.9,10.9], weight N(0,sqrt2)+3e-4 floor
SEED_A = 0.26290939635800314
SEED_C = 1.3681225894947517
SEED_D = 1.8829810106831677
COEF = [
    1.2262729945630335,
    0.35021806233529135,
    0.13069888011324796,
    0.030719212594285544,
    0.06172033167075067,
]
MAGIC = float(1.5 * 2**23)

# feature tiles per side: 0 sinA(1) 1 cosA(1) 2 s3(-1/4) 3 c3(1/4)
#   4 sinC(1) 5 cosC(1) 6 s2c(1/2) 7 c2c(1/2) 8 sinD(1) 9 cosD(1)
AMP = [1.0, 1.0, -0.25, 0.25, 1.0, 1.0, 0.5, 0.5, 1.0, 1.0]
ROWS = []
for fi in range(5):
    si, ci = 2 * fi, 2 * fi + 1
    ROWS.append((si, ci, COEF[fi] / (AMP[si] * AMP[ci])))
    ROWS.append((ci, si, COEF[fi] / (AMP[ci] * AMP[si])))
NT = len(ROWS)  # 10

_CACHE: dict = {}


def _build_nc():
    import concourse.bacc as bacc
    import concourse.tile as tile
    from concourse import mybir

    f32 = mybir.dt.float32
    bf16 = mybir.dt.bfloat16
    AF = mybir.ActivationFunctionType
    ALU = mybir.AluOpType
    PI = float(np.pi)

    nc = bacc.Bacc("TRN2", target_bir_lowering=False, debug=False)

    qT_d = nc.dram_tensor("qT", [H, PTS], f32, kind="ExternalInput")
    kT_d = nc.dram_tensor("kT", [H, PTS], f32, kind="ExternalInput")
    vaug_d = nc.dram_tensor("vaug", [128, BPC * 2 * 129], bf16, kind="ExternalInput")
    wa_d = nc.dram_tensor("wa", [128, NT], f32, kind="ExternalInput")
    out_d = nc.dram_tensor("out", [128, BPC * 2 * H], bf16, kind="ExternalOutput")

    with tile.TileContext(nc) as tc:
        with (
            tc.tile_pool(name="const", bufs=1) as cpool,
            tc.tile_pool(name="qf", bufs=1) as qf_pool,
            tc.tile_pool(name="ksw", bufs=1) as ksw_pool,
            tc.tile_pool(name="kraw", bufs=1) as kraw_pool,
            tc.tile_pool(name="tmp", bufs=1) as tmp_pool,
            tc.tile_pool(name="tmpf", bufs=2) as tmpf_pool,
            tc.tile_pool(name="expS", bufs=4) as expS_pool,
            tc.tile_pool(name="osb", bufs=3) as out_pool,
            tc.tile_pool(name="small", bufs=4) as small_pool,
            tc.tile_pool(name="ps", bufs=8, space="PSUM") as ps_pool,
        ):
            kT = cpool.tile([H, PTS], f32, tag="kT")
            nc.sync.dma_start(kT[:], kT_d.ap()[:, :])
            qT = cpool.tile([H, PTS], f32, tag="qT")
            nc.scalar.dma_start(qT[:], qT_d.ap()[:, :])
            wa = cpool.tile([128, NT], f32, tag="wa")
            nc.gpsimd.dma_start(wa[:], wa_d.ap()[:, :])
            vaug = cpool.tile([128, BPC * 2 * 129], bf16, tag="vaug")
            nc.gpsimd.dma_start(vaug[:], vaug_d.ap()[:, :])

            halfpi = cpool.tile([128, 1], f32, tag="halfpi")
            nc.vector.memset(halfpi[:], PI / 2)

            def wcol(t):
                return wa[:, t : t + 1]

            def quad(src, seed, side, u_on_act=False):
                """Range reduction: returns (f, fc) fp32 tiles for one seed."""
                invP = float(seed / (2 * PI))
                u = tmp_pool.tile([H, PTS], f32, name="u")
                if u_on_act:
                    # Copy keeps float-imm bias: u = src*invP + MAGIC on ACT
                    nc.scalar.activation(u[:], src, AF.Copy, bias=MAGIC, scale=invP)
                else:
                    nc.vector.tensor_scalar(u[:], src, invP, MAGIC, ALU.mult, ALU.add)
                n = tmp_pool.tile([H, PTS], f32, name="n")
                nc.vector.tensor_scalar_sub(n[:], u[:], MAGIC)
                f = tmpf_pool.tile([H, PTS], f32, name="f")
                nc.vector.scalar_tensor_tensor(f[:], src, invP, n[:], ALU.mult, ALU.subtract)
                fc = tmpf_pool.tile([H, PTS], f32, name="fc")
                nc.vector.tensor_scalar(
                    fc[:].bitcast(mybir.dt.uint32),
                    f[:].bitcast(mybir.dt.uint32),
                    0x7FFFFFFF,
                    None,
                    ALU.bitwise_and,
                )
                return f, fc

            # ================= K-side range reductions ======================
            kf_c, kfc_c = quad(kT[:], SEED_C, "k")
            kf_d, kfc_d = quad(kT[:], SEED_D, "k", u_on_act=True)

            # ================= K side =======================================
            # ACT sins (bf16)
            k_sA = kraw_pool.tile([H, PTS], bf16, name="k_sA")
            nc.scalar.activation(k_sA[:], kT[:], AF.Sin, scale=SEED_A)
            k_cA = kraw_pool.tile([H, PTS], bf16, name="k_cA")
            nc.scalar.activation(k_cA[:], kT[:], AF.Sin, bias=halfpi[:], scale=SEED_A)
            k_sC = kraw_pool.tile([H, PTS], bf16, name="k_sC")
            nc.scalar.activation(k_sC[:], kf_c[:], AF.Sin, scale=2 * PI)
            k_cC = kraw_pool.tile([H, PTS], bf16, name="k_cC")
            nc.scalar.activation(k_cC[:], kfc_c[:], AF.Sin, bias=halfpi[:], scale=-2 * PI)
            k_sD = kraw_pool.tile([H, PTS], bf16, name="k_sD")
            nc.scalar.activation(k_sD[:], kf_d[:], AF.Sin, scale=2 * PI)
            k_cD = kraw_pool.tile([H, PTS], bf16, name="k_cD")
            nc.scalar.activation(k_cD[:], kfc_d[:], AF.Sin, bias=halfpi[:], scale=-2 * PI)

            ksw = [None] * NT

            def ktile(t):
                ksw[t] = ksw_pool.tile([H, PTS], bf16, name=f"ksw{t}")
                return ksw[t]

            # Pool: raw squares only
            k_sA2 = tmp_pool.tile([H, PTS], f32, name="k_sA2")
            nc.gpsimd.tensor_mul(k_sA2[:], k_sA[:], k_sA[:])
            k_cA2 = tmp_pool.tile([H, PTS], f32, name="k_cA2")
            nc.gpsimd.tensor_mul(k_cA2[:], k_cA[:], k_cA[:])
            k_cS2 = tmp_pool.tile([H, PTS], f32, name="k_cS2")
            nc.gpsimd.tensor_mul(k_cS2[:], k_cC[:], k_cC[:])

            qfeat = [None] * NT

            def qtile(i):
                qfeat[i] = qf_pool.tile([H, PTS], bf16, name=f"qf{i}")
                return qfeat[i]

            # DVE: rows t0/t1 as soon as the k seed-a sins land
            nc.vector.tensor_scalar_mul(ktile(0)[:], k_cA[:], wcol(0))
            nc.vector.tensor_scalar_mul(ktile(1)[:], k_sA[:], wcol(1))

            # q-side reductions + sins
            qf_c, qfc_c = quad(qT[:], SEED_C, "q", u_on_act=True)
            nc.scalar.activation(qtile(0)[:], qT[:], AF.Sin, scale=SEED_A)
            nc.scalar.activation(qtile(1)[:], qT[:], AF.Sin, bias=halfpi[:], scale=SEED_A)
            nc.scalar.activation(qtile(4)[:], qf_c[:], AF.Sin, scale=2 * PI)
            nc.scalar.activation(qtile(5)[:], qfc_c[:], AF.Sin, bias=halfpi[:], scale=-2 * PI)

            # DVE: rows t4/t5
            nc.vector.tensor_scalar_mul(ktile(4)[:], k_cC[:], wcol(4))
            nc.vector.tensor_scalar_mul(ktile(5)[:], k_sC[:], wcol(5))

            # Pool: k s2 product + q squares
            k_s2raw = tmp_pool.tile([H, PTS], bf16, name="k_s2raw")
            nc.gpsimd.tensor_mul(k_s2raw[:], k_sC[:], k_cC[:])
            q_sA2 = tmp_pool.tile([H, PTS], f32, name="q_sA2")
            nc.gpsimd.tensor_mul(q_sA2[:], qfeat[0][:], qfeat[0][:])
            q_cA2 = tmp_pool.tile([H, PTS], f32, name="q_cA2")
            nc.gpsimd.tensor_mul(q_cA2[:], qfeat[1][:], qfeat[1][:])

            # DVE: rows t3/t2 (k triples)
            sAw = tmp_pool.tile([H, PTS], bf16, name="sAw")
            nc.vector.tensor_scalar_mul(sAw[:], k_sA[:], wcol(3))
            cAw = tmp_pool.tile([H, PTS], bf16, name="cAw")
            nc.vector.tensor_scalar_mul(cAw[:], k_cA[:], wcol(2))
            nc.vector.scalar_tensor_tensor(ktile(3)[:], k_sA2[:], 0.75, sAw[:], ALU.subtract, ALU.mult)
            nc.vector.scalar_tensor_tensor(ktile(2)[:], k_cA2[:], 0.75, cAw[:], ALU.subtract, ALU.mult)

            # q-side seed d reduction + sins
            qf_d, qfc_d = quad(qT[:], SEED_D, "q", u_on_act=True)
            nc.scalar.activation(qtile(8)[:], qf_d[:], AF.Sin, scale=2 * PI)
            nc.scalar.activation(qtile(9)[:], qfc_d[:], AF.Sin, bias=halfpi[:], scale=-2 * PI)

            # Pool: q c-double square, then qf6 product
            q_cS2 = tmp_pool.tile([H, PTS], f32, name="q_cS2")
            nc.gpsimd.tensor_mul(q_cS2[:], qfeat[5][:], qfeat[5][:])
            nc.gpsimd.tensor_mul(qtile(6)[:], qfeat[4][:], qfeat[5][:])

            # DVE: remaining rows
            nc.vector.tensor_scalar_mul(ktile(7)[:], k_s2raw[:], wcol(7))
            nc.vector.tensor_scalar(ktile(6)[:], k_cS2[:], 0.5, wcol(6), ALU.subtract, ALU.mult)
            nc.vector.scalar_tensor_tensor(qtile(2)[:], q_sA2[:], 0.75, qfeat[0][:], ALU.subtract, ALU.mult)
            nc.vector.scalar_tensor_tensor(qtile(3)[:], q_cA2[:], 0.75, qfeat[1][:], ALU.subtract, ALU.mult)
            nc.vector.tensor_scalar_mul(ktile(8)[:], k_cD[:], wcol(8))
            nc.vector.tensor_scalar_mul(ktile(9)[:], k_sD[:], wcol(9))
            nc.vector.tensor_scalar_sub(qtile(7)[:], q_cS2[:], 0.5)

            # ================= scores =======================================
            def psum_bank():
                return ps_pool.tile([128, 2 * Q], f32, name="psb")

            scores_ps = {}
            for b in range(BPC):
                for chunk in range(2):
                    scores_ps[(b, chunk)] = psum_bank()
            MM_ORDER = [0, 1, 4, 5, 2, 3, 6, 8, 9, 7]
            for mi, t in enumerate(MM_ORDER):
                qi, ki, _ = ROWS[t]
                for b in range(BPC):
                    for chunk in range(2):
                        nc.tensor.matmul(
                            scores_ps[(b, chunk)][:, 0:Q],
                            ksw[t][:, b * K + chunk * 128 : b * K + chunk * 128 + 128],
                            qfeat[qi][:, b * Q : (b + 1) * Q],
                            start=(mi == 0),
                            stop=(mi == NT - 1),
                        )

            # ================= softmax + out ================================
            outs = []
            for b in range(BPC):
                eS = {}
                for chunk in range(2):
                    e = expS_pool.tile([128, Q], bf16, name="eS")
                    nc.scalar.activation(e[:], scores_ps[(b, chunk)][:, 0:Q], AF.Exp)
                    eS[chunk] = e
                for qb in range(2):
                    outp = psum_bank()
                    for chunk in range(2):
                        nc.tensor.matmul(
                            outp[:, 0:129],
                            eS[chunk][:, qb * 128 : qb * 128 + 128],
                            vaug[:, (b * 2 + chunk) * 129 : (b * 2 + chunk + 1) * 129],
                            start=(chunk == 0),
                            stop=(chunk == 1),
                        )
                    outs.append((b, qb, outp))
            osb_all = cpool.tile([128, BPC * 2 * H], bf16, tag="osb_all")
            for gi, (b, qb, outp) in enumerate(outs):
                recip = small_pool.tile([128, 1], f32)
                nc.vector.reciprocal(recip[:], outp[:, 128:129])
                g = b * 2 + qb
                dst = osb_all[:, g * H : (g + 1) * H]
                nc.vector.tensor_scalar_mul(dst, outp[:, 0:128], recip[:])
            HALF = BPC * H  # groups 0-3 | 4-7
            nc.sync.dma_start(out_d.ap()[:, 0:HALF], osb_all[:, 0:HALF])
            nc.sync.dma_start(out_d.ap()[:, HALF:], osb_all[:, HALF:])

    nc.compile()
    return nc


def _get_nc():
    if "nc" not in _CACHE:
        _CACHE["nc"] = _build_nc()
    return _CACHE["nc"]


def _prep_core_inputs(queries, keys, values, w, c):
    import ml_dtypes

    bs = slice(c * BPC, (c + 1) * BPC)
    qT = np.ascontiguousarray(
        queries[bs].transpose(2, 0, 1).reshape(H, PTS), dtype=np.float32
    )
    kT = np.ascontiguousarray(
        keys[bs].transpose(2, 0, 1).reshape(H, PTS), dtype=np.float32
    )
    va = np.ones((BPC, 2, 128, 129), dtype=np.float32)
    va[..., :128] = values[bs].reshape(BPC, 2, 128, 128)
    vaug = np.ascontiguousarray(
        va.transpose(2, 0, 1, 3).reshape(128, BPC * 2 * 129)
    ).astype(ml_dtypes.bfloat16)
    wa = np.empty((128, NT), dtype=np.float32)
    for t, (qi, ki, cf) in enumerate(ROWS):
        wa[:, t] = w * np.float32(cf)
    return {"qT": qT, "kT": kT, "vaug": vaug, "wa": wa}


def kernel(queries, keys, values, w):
    from concourse.bass_utils import run_bass_kernel_spmd
    from concourse._compat import axon_active

    if os.environ.get("BASS_TRACE") and axon_active():
        try:
            import antenv.axon_hooks  # noqa: F401
        except ImportError:
            os.environ["BASS_NEVER_TRACE"] = "1"

    queries = np.asarray(queries, dtype=np.float32)
    keys = np.asarray(keys, dtype=np.float32)
    values = np.asarray(values, dtype=np.float32)
    w = np.asarray(w, dtype=np.float32)

    nc = _get_nc()
    in_maps = [_prep_core_inputs(queries, keys, values, w, c) for c in range(NCORES)]
    res = run_bass_kernel_spmd(nc, in_maps, core_ids=list(range(NCORES)))
    _CACHE["last_result"] = res
    outs = []
    for c in range(NCORES):
        o = np.asarray(res.results[c]["out"], dtype=np.float32)  # [128, BPC*2*H]
        o = o.reshape(128, BPC * 2, H).transpose(1, 0, 2).reshape(BPC * Q, H)
        outs.append(o)
    out = np.concatenate(outs, axis=0)
    return out.reshape(B, Q, H)
